# revision 33
# baseline (speedup 1.0000x reference)
"""Trainium2 Bass kernel for nn_CCM: per-pixel complex 3x3 conv with
mask-derived weights.

out(b,t,f) complex = sum_{i,j} H[i,j](b,t,f) * x(b, t+i-2, f+j-1)
  H_real[k] = m0k - 0.5*(m1k + m2k);  H_imag[k] = (sqrt(3)/2)*(m1k - m2k)
where m channel c = r*9 + k, k = i*3 + j.

Sharding: pure data-parallel over batch B=8 -> one batch element per core.
Layout on-chip: partition dim = t (chunks of 128), free dim = f. x is
transposed (F,T,2)->(t,f) on the TensorEngine with the 3 causal t-shifts
materialized as free-dim slices, so every tap is a free-dim offset view.
Products run on DVE with multi-axis APs; tap accumulation is an in-place
binary tree.

The axon/PJRT walrus codegen allows at most ONE sync wait per instruction.
Strategy: all DMAs are dispatched from ACT (so their data deps are already
observed on ACT and each DMA carries only its non-elidable DMA-lane wait,
which coincides with its buffer-WAW target at 4 DMAs/chunk x 8 lanes);
every other cross-engine or completion hazard is pre-observed by real
single-wait carrier ops (ldweights on PE, unique-tile memsets on DVE/ACT)
wired with add_dep_helper; PE/DVE/ACT streams are order-pinned (Seq).
"""

import sys

sys.path.insert(0, "/opt/trn_rl_repo")

import numpy as np
from contextlib import ExitStack

import concourse.bass as bass
import concourse.tile as tile
from concourse import mybir
from concourse.masks import make_identity
from concourse import bass_utils
from bass_rust import add_dep_helper

F32 = mybir.dt.float32
T, F, C = 1000, 257, 27
NCORES = 8
CHUNK = 128
CHUNKS = [(t0, min(CHUNK, T - t0)) for t0 in range(0, T, CHUNK)]
FBLOCKS = [(0, 86), (86, 86), (172, 85)]
W = 260  # x-shift tile width: col 0 = f=-1 guard, 1..257 = f, 258 = f=257 guard
RT3_2 = float(np.sqrt(3.0) / 2.0)
ADD = mybir.AluOpType.add
SUB = mybir.AluOpType.subtract
MULT = mybir.AluOpType.mult


def _view(t_ap, tw, s0, n_slots, fdim=F):
    """[tw, n_slots/3, 3, F] view over slot axis of a [128, S, F] tile."""
    sl = t_ap[0:tw, s0:s0 + n_slots, :]
    return bass.AP(
        tensor=sl.tensor,
        offset=sl.offset,
        ap=[sl.ap[0], [3 * fdim, n_slots // 3], [fdim, 3], [1, fdim]],
    )


def _xview(xs, tw):
    """Overlapping tap view [tw, 3(i), 3(j), F] over a [128, 3, W] shift tile."""
    sl = xs[0:tw]
    return bass.AP(
        tensor=sl.tensor,
        offset=sl.offset,
        ap=[sl.ap[0], [W, 3], [1, 3], [1, F]],
    )


def _dep(frm, *tos):
    for to in tos:
        if to is not None:
            add_dep_helper(frm.ins, to.ins, sync=True, reason="carrier dep")
    return frm


class Seq:
    """Pins an engine's instructions to emission order so wait-elision sees
    a deterministic observed-semaphore history."""

    def __init__(self):
        self.last = {}

    def __call__(self, key, inst):
        prev = self.last.get(key)
        if prev is not None:
            add_dep_helper(inst.ins, prev.ins, sync=False, reason="engine order")
        self.last[key] = inst
        return inst


def build_nc():
    nc = bass.Bass()
    m = nc.declare_dram_parameter("m", [C, T, F], F32, isOutput=False)
    x = nc.declare_dram_parameter("x", [F, T, 2], F32, isOutput=False)
    out = nc.declare_dram_parameter("out", [F, T, 2], F32, isOutput=True)
    seq = Seq()

    with ExitStack() as ctx:
        tc = ctx.enter_context(tile.TileContext(nc))
        consts = ctx.enter_context(tc.tile_pool(name="consts", bufs=1))
        mpool = ctx.enter_context(tc.tile_pool(name="mpool", bufs=2))
        hdpool = ctx.enter_context(tc.tile_pool(name="hdpool", bufs=2))
        xshpool = ctx.enter_context(tc.tile_pool(name="xshpool", bufs=2))
        prodpool = ctx.enter_context(tc.tile_pool(name="prodpool", bufs=1))
        outpool = ctx.enter_context(tc.tile_pool(name="outpool", bufs=2))
        xpsum = ctx.enter_context(tc.tile_pool(name="xpsum", bufs=1, space="PSUM"))
        opsum = ctx.enter_context(tc.tile_pool(name="opsum", bufs=1, space="PSUM"))

        # --- prologue ---
        ident_gp = consts.tile([128, 128], F32, name="ident_gp")
        make_identity(nc, ident_gp)
        # a separate last Pool op we hold a handle to ([0,0] is 1.0 anyway)
        ident_gp_done = nc.gpsimd.memset(ident_gp[0:1, 0:1], 1.0)
        ident = consts.tile([128, 128], F32, name="ident")
        seq("a", nc.scalar.copy(out=ident, in_=ident_gp))
        wdum = consts.tile([2, 2], mybir.dt.bfloat16, name="wdum")
        seq("a", nc.scalar.memzero(wdum))

        ncar = [0]

        def dve_car(*deps):
            tcar = consts.tile([1, 8], F32, name=f"car{ncar[0]}",
                               tag=f"car{ncar[0]}")
            ncar[0] += 1
            return seq("v", _dep(nc.vector.memset(tcar, 0.0), *deps))

        def act_car(*deps):
            tcar = consts.tile([1, 8], F32, name=f"car{ncar[0]}",
                               tag=f"car{ncar[0]}")
            ncar[0] += 1
            return seq("a", _dep(nc.scalar.memzero(tcar), *deps))

        def pe_car(*deps):
            return seq("p", _dep(nc.tensor.ldweights(weights=wdum[0:2, 0:2]),
                                 *deps))

        # x staged with 2 leading zero t-columns: col t' = t + 2, so every
        # causal shift window (t0 + i - 2 ..) maps to a valid source range.
        # All DMAs dispatch from ACT (see module docstring).
        all_dmas = []
        xorig = []
        prev_xs_acts = []
        for fi, (f0, fw) in enumerate(FBLOCKS):
            xst = consts.tile([fw, T, 2], F32, name=f"xst{fi}", tag=f"xst{fi}")
            xdma = seq("a", nc.scalar.dma_start(out=xst, in_=x[f0:f0 + fw, :, :]))
            all_dmas.append(xdma)
            xo = consts.tile([fw, T + 2, 2], F32, name=f"xorig{fi}",
                             tag=f"xorig{fi}")
            seq("a", nc.scalar.memzero(xo[:, 0:2, :]))
            prev_xs_acts.append(
                seq("a", nc.scalar.copy(out=xo[:, 2:T + 2, :], in_=xst)))
            xorig.append(xo)

        prev_pack_acts = []
        prev_out_trs = []
        prev_store_dmas = []
        old_store_dmas = []
        last_act = None   # last ACT op of previous chunk (self-hazard cover)
        last_dve = None   # last DVE op of previous chunk
        last_pe = None    # last PE op of previous chunk

        for ci, (t0, tw) in enumerate(CHUNKS):
            # ACT self/foreign pre-observers for this chunk
            if last_act is not None:
                act_car(last_act)          # ACT self-completion (old slots)
            if prev_out_trs:
                act_car(prev_out_trs[-1])  # PE tick: er2/ei2 + pk slots free
            for sd in old_store_dmas:
                act_car(sd)                # store lanes: pks slots free
            old_store_dmas = prev_store_dmas
            prev_store_dmas = []

            # --- single m load: all 27 channels -> [tw, 27, F] ---
            mt = mpool.tile([128, C, F], F32, name="mt", tag="mt")
            mdma = seq("a", nc.scalar.dma_start(
                out=mt[0:tw],
                in_=m[:, t0:t0 + tw, :].rearrange("k t f -> t k f"),
            ))
            all_dmas.append(mdma)

            # --- x transposes: (f,t) -> (t,f) with 3 causal t-shifts ---
            pe_car(*prev_xs_acts)                  # ACT tick (xorig/xs ready)
            if last_pe is not None:
                pe_car(last_pe)                    # PE self (PSUM slot WAW)
            prev_xs_acts = []
            xsh = []
            for ch in range(2):
                # flat PSUM tile; i-slices bank-aligned at 512-f32 offsets so
                # no transpose output crosses a 2KB PSUM bank boundary
                xp = xpsum.tile([128, 1536], F32, name=f"xps{ch}", tag=f"xps{ch}")
                for i in range(3):
                    for (f0, fw), xo in zip(FBLOCKS, xorig):
                        base = 512 * i + 1 + f0
                        seq("p", nc.tensor.transpose(
                            xp[0:tw, base:base + fw],
                            xo[:, t0 + i:t0 + i + tw, ch],
                            ident[0:fw, 0:fw],
                        ))
                xs = xshpool.tile([128, 3, W], F32, name=f"xsh{ch}", tag=f"xsh{ch}")
                # copy only the transpose-written cols; zero the f-guard cols
                xp_v = bass.AP(tensor=xp.tensor, offset=xp.offset + 1,
                               ap=[xp[0:tw].ap[0], [512, 3], [1, 257]])
                prev_xs_acts.append(seq("a", nc.scalar.copy(
                    out=xs[0:tw, :, 1:258], in_=xp_v)))
                prev_xs_acts.append(
                    seq("a", nc.scalar.memzero(xs[0:tw, :, 0:1])))
                prev_xs_acts.append(
                    seq("a", nc.scalar.memzero(xs[0:tw, :, 258:260])))
                xsh.append(xs)
            xrv = _xview(xsh[0], tw)
            xiv = _xview(xsh[1], tw)

            # --- H on DVE: s = m1+m2 (->Hr in place), d = m1-m2 ---
            if last_dve is not None:
                dve_car(last_dve)                  # DVE self (slot WAWs)
            m0 = mt[0:tw, 0:9]
            m1 = mt[0:tw, 9:18]
            m2 = mt[0:tw, 18:27]
            s = hdpool.tile([128, 9, F], F32, name="s_t", tag="s_t")
            d = hdpool.tile([128, 9, F], F32, name="d_t", tag="d_t")
            seq("v", nc.vector.tensor_add(s[0:tw], m1, m2))
            seq("v", nc.vector.tensor_sub(d[0:tw], m1, m2))
            hr = s
            seq("v", nc.vector.scalar_tensor_tensor(
                out=hr[0:tw], in0=s[0:tw], scalar=-0.5, in1=m0,
                op0=MULT, op1=ADD,
            ))
            hrv = _view(hr, tw, 0, 9)

            dve_car(*prev_xs_acts)                 # ACT tick (xs tiles ready)

            # --- products: [tw, 18, F] per output channel ---
            per = prodpool.tile([128, 18, F], F32, name="per", tag="per")
            pei = prodpool.tile([128, 18, F], F32, name="pei", tag="pei")
            seq("v", nc.vector.tensor_tensor(_view(per, tw, 0, 9), hrv, xrv, MULT))
            seq("v", nc.vector.tensor_tensor(_view(pei, tw, 0, 9), hrv, xiv, MULT))
            # STT is limited to 3D APs -> one op per i-shift
            for i in range(3):
                xrv_i = bass.AP(tensor=xsh[0].tensor, offset=xsh[0].offset + i * W,
                                ap=[xsh[0][0:tw].ap[0], [1, 3], [1, F]])
                xiv_i = bass.AP(tensor=xsh[1].tensor, offset=xsh[1].offset + i * W,
                                ap=[xsh[1][0:tw].ap[0], [1, 3], [1, F]])
                seq("v", nc.vector.scalar_tensor_tensor(
                    out=per[0:tw, 9 + 3 * i:12 + 3 * i],
                    in0=d[0:tw, 3 * i:3 * i + 3],
                    scalar=-RT3_2, in1=xiv_i, op0=MULT, op1=MULT,
                ))
                seq("v", nc.vector.scalar_tensor_tensor(
                    out=pei[0:tw, 9 + 3 * i:12 + 3 * i],
                    in0=d[0:tw, 3 * i:3 * i + 3],
                    scalar=RT3_2, in1=xrv_i, op0=MULT, op1=MULT,
                ))

            # --- tap-sum: in-place binary tree over the 18 slots ---
            ero = outpool.tile([128, F], F32, name="ero", tag="ero")
            eio = outpool.tile([128, F], F32, name="eio", tag="eio")
            tree_last = None
            for p, acc in ((per, ero), (pei, eio)):
                seq("v", nc.vector.tensor_add(p[0:tw, 0:9], p[0:tw, 0:9],
                                              p[0:tw, 9:18]))
                seq("v", nc.vector.tensor_add(p[0:tw, 0:4], p[0:tw, 0:4],
                                              p[0:tw, 4:8]))
                seq("v", nc.vector.tensor_add(p[0:tw, 0:2], p[0:tw, 0:2],
                                              p[0:tw, 2:4]))
                seq("v", nc.vector.tensor_add(p[0:tw, 0:1], p[0:tw, 0:1],
                                              p[0:tw, 1:2]))
                tree_last = seq("v", nc.vector.tensor_add(
                    acc[0:tw], p[0:tw, 0, :], p[0:tw, 8, :]))
            last_dve = tree_last

            # --- transpose back to (f, t), pack (t,2), store ---
            er2 = outpool.tile([128, F], F32, name="er2", tag="er2")
            ei2 = outpool.tile([128, F], F32, name="ei2", tag="ei2")
            cr = seq("a", nc.scalar.copy(out=er2[0:tw], in_=ero[0:tw]))
            ci = seq("a", nc.scalar.copy(out=ei2[0:tw], in_=eio[0:tw]))
            pe_car(cr, ci, *prev_pack_acts)
            prev_pack_acts = []
            prev_out_trs = []
            packs = []
            pk = opsum.tile([128, 2, 3, 128], F32, name="pk", tag="pk")
            for fbi, (f0, fw) in enumerate(FBLOCKS):
                prev_out_trs.append(seq("p", nc.tensor.transpose(
                    pk[0:fw, 0, fbi, 0:tw], er2[0:tw, f0:f0 + fw],
                    ident[0:tw, 0:tw])))
                prev_out_trs.append(seq("p", nc.tensor.transpose(
                    pk[0:fw, 1, fbi, 0:tw], ei2[0:tw, f0:f0 + fw],
                    ident[0:tw, 0:tw])))
                pks = outpool.tile([128, 128, 2], F32, name="pks", tag="pks",
                                   bufs=6)
                pks_v = bass.AP(
                    tensor=pks.tensor, offset=pks.offset,
                    ap=[pks[0:fw].ap[0], [1, 2], [2, tw]],
                )
                pack = seq("a", nc.scalar.copy(out=pks_v,
                                                in_=pk[0:fw, :, fbi, 0:tw]))
                prev_pack_acts.append(pack)
                # pre-observe the pack's completion on ACT, then dispatch
                ac = act_car(pack)
                sd = seq("a", nc.scalar.dma_start(
                    out=out[f0:f0 + fw, t0:t0 + tw, :], in_=pks[0:fw, 0:tw]))
                packs.append(sd)
                prev_store_dmas.append(sd)
                all_dmas.append(sd)
            last_act = ac
            last_pe = prev_out_trs[-1]

        # --- epilogue: pre-observe every proc's final tick on SP via
        # register-move carriers (no memory access -> no extra hazards), so
        # the TileContext exit drain's waits all elide (strict codegens cap
        # an instruction's sync waits).
        last_per_lane = {}
        for idx, dma in enumerate(all_dmas):
            last_per_lane[idx % 8] = dma
        spr = nc.sync.alloc_register(name="spcar")
        finals = list(last_per_lane.values()) + [
            last_act, last_dve, last_pe, prev_out_trs[-1], ident_gp_done]
        for dep in finals:
            if dep is None:
                continue
            seq("s", _dep(nc.sync.reg_mov(spr, 0), dep))
    return nc


_NC = None


def _get_nc():
    global _NC
    if _NC is None:
        _NC = build_nc()
    return _NC


def kernel(m, x):
    assert m.shape == (NCORES, C, T, F) and x.shape == (NCORES, F, T, 2)
    nc = _get_nc()
    in_maps = [
        {"m": np.ascontiguousarray(m[b]), "x": np.ascontiguousarray(x[b])}
        for b in range(NCORES)
    ]
    res = bass_utils.run_bass_kernel_spmd(nc, in_maps, core_ids=list(range(NCORES)))
    return np.stack([res.results[b]["out"] for b in range(NCORES)], axis=0)
